# revision 30
# baseline (speedup 1.0000x reference)
"""NeRD pixel decoder (SIREN MLP over 5x5 local patches) on 8 trn2 cores.

Sharding: row-shard the pixel dim. Core c handles image b=c//4, rows
y0=(c%4)*32 .. y0+32 (4096 pixels). SIREN weights replicated.

Layer 0 (the 5x5 conv, 84% of FLOPs) runs in fp8-e4m3 DoubleRow matmuls at
0.5 cycles/row: per output row and 128-out-chan block, 25 taps are computed
as DR pairs (x_hi, x_lo) against stride-0-duplicated fp8 weights (x split
into hi + lo e4m3 parts on host, recovering ~11-bit input precision), plus 8
weight-residual correction DR pairs: 7 over vertically adjacent taps (their
windows don't overlap -- overlapping DR rhs windows crash the PE) and one
mixed pair whose halves are (w_lo of tap12) and the coords contribution
(gx/gy baked into a third slab plane at the same in-plane offset as tap12,
so the pair's two-dim stride is exactly 2*SLAB). Layers 1/2 (tiles 0-6)
also run fp8-e4m3 DoubleRow: the DVE splits each bf16 h tile into e4m3
hi/lo planes (cast + subtract, ~0.3us per 256-px slice on an otherwise
idle engine), and each 512-px psum bank takes two 256-px chains of three
DR matmuls (two stride-0 hi/lo k-mains + one w_lo correction pair) --
768 cycles instead of 1024. The head and the drain tile stay in bf16
weights x bf16 activations (1 cycle/row at any moving size, unlike f32r's
4x penalty under 256), which makes the 128-px drain sub-tiles cheap.

Pipeline: ~12 input DMAs (hi/lo/coords slabs ride one tensor in row chunks;
b0 rides w0l via byte-packing; w1/w2/w3 ride one bf16 tensor; b1/b2/b3 one
tiny f32 tensor), all on the SP queue in strict first-use order -- each
dma_start costs ~650ns of sequencer + HWDGE issue and transfers serialize,
so order is everything and fewer is faster. Output DMAs also go on the SP
queue: a dma_start dispatched from the ACT engine blocks later activations
on that sequencer. Dummy fp8 warmup matmuls on a memset scratch cover the
DMA lead-in and the PE p-state ramp (full clock needs ~3us of continuous
execution). m=1 row units lag m=0 by two rows so the m=1 weight DMA can
land later. L1/L2/head tiles are fused into the L0 row stream with a
one-tile stagger, and the final tile drains through 256/128-px sub-tiles
with filler matmuls between pipeline links, so cross-engine dependencies
are old when the PE reaches them and the drain tail is short.

Everything is quantized host-side (e4m3 via ml_dtypes, f32r/bf16 rounding);
the device only multiplies exactly and accumulates in f32 PSUM. Weight
scale 2^12 and x scale 2^2 keep e4m3 operands in normal range; the
activation scale folds 2^-14 back out (sin(OMEGA*(z+b0)) via ACT bias).

Measured on the 8-core axon trn2 setup: TimelineSim 82044 ns (sim matched
HW within 2% on the 147030 ns baseline), rel err 1.83e-2 vs the fp32
reference (gate 2e-2; fully deterministic for the fixed seed-0 inputs).
Dense layers emit k-major so the freshest cross-engine dependency (the
just-written h half) is read by the last matmul of the group, not the
first.
"""

import numpy as np
import ml_dtypes

FC = 128      # feature channels
P = 5         # patch
HID = 256
OUT = 3
OMEGA = 30.0
B, H, W = 2, 128, 128
NCORES = 8
ROWS = H // 4            # 32 image rows per core
NPIX = ROWS * W          # 4096 pixels per core
SLABR = ROWS + 4         # 36 slab rows (2 halo each side)
SLABW = W + 4            # 132 slab cols (2 pad each side)
SLAB = SLABR * SLABW     # 4752
TP = 512                 # pixels per L1/L2/head PSUM tile (= 4 image rows)
NT = NPIX // TP          # 8 tiles per core

E4 = ml_dtypes.float8_e4m3
SX = 4.0                 # x (slab/coords) pre-scale
SWT = 4096.0             # layer-0 weight pre-scale
SW12 = 64.0              # L1/L2 fp8 weight pre-scale
NCORR = 15               # w_lo-corrected taps: 7 vertical pairs + tap12 in the
                         # mixed pair whose second half is the coords plane
NWARM = 72               # warmup DR matmuls during DMA lead-in
WPAIRS = [(dx, q) for q in range(2) for dx in range(5)][:7]  # 14 taps; +tap12 mixed
WLBLK = 8 * 256          # per-m w0l bytes: 8 DR pair blocks
WTAIL = 5                # packed b1|b2|b3 columns (f32)
WBC = 1030               # packed w1|w2|w3 columns (bf16)

_BUILT = {}


def _build(structure="v8"):
    key = structure
    if key in _BUILT:
        return _BUILT[key]

    import concourse.tile as tile
    import concourse.mybir as mybir
    from concourse import bacc

    f32 = mybir.dt.float32
    f32r = mybir.dt.float32r
    fp8 = mybir.dt.float8e4
    Sin = mybir.ActivationFunctionType.Sin
    DR = mybir.MatmulPerfMode.DoubleRow

    nc = bacc.Bacc("TRN2", target_bir_lowering=False, debug=False)

    xs = nc.dram_tensor("xs", [128, 3 * SLAB], fp8, kind="ExternalInput").ap()
    w0h = nc.dram_tensor("w0h", [128, 6400], fp8,
                         kind="ExternalInput").ap()
    w0l = nc.dram_tensor("w0l", [128, 8 + 2 * WLBLK], fp8,
                         kind="ExternalInput").ap()
    wt = nc.dram_tensor("wt", [128, WTAIL], f32, kind="ExternalInput").ap()
    wb = nc.dram_tensor("wb", [128, WBC], mybir.dt.bfloat16,
                        kind="ExternalInput").ap()
    wq = nc.dram_tensor("wq", [128, 2048], fp8, kind="ExternalInput").ap()
    out = nc.dram_tensor("out", [OUT, NPIX], f32, kind="ExternalOutput").ap()

    with tile.TileContext(nc) as tc:
        with (
            tc.tile_pool(name="const", bufs=1) as cpool,
            tc.tile_pool(name="h", bufs=3) as hpool,
            tc.tile_pool(name="osb", bufs=1) as opool,
            tc.tile_pool(name="ps", bufs=8, space="PSUM") as pspool,
        ):
            # ---- SBUF tiles ----
            xs_t = cpool.tile([128, 3 * SLAB], fp8, tag="xs", name="xs_t")
            w0h_t = cpool.tile([128, 6400], fp8, tag="w0h", name="w0h_t")
            w0l_t = cpool.tile([128, 8 + 2 * WLBLK], fp8, tag="w0l",
                               name="w0l_t")
            wt_t = cpool.tile([128, WTAIL], f32, tag="wt", name="wt_t")
            wb_t = cpool.tile([128, WBC], mybir.dt.bfloat16, tag="wb",
                              name="wb_t")
            wq_t = cpool.tile([128, 2048], fp8, tag="wq", name="wq_t")
            h0q = cpool.tile([128, 4 * NPIX], fp8, tag="h0q", name="h0q")
            h1q = cpool.tile([128, 4 * NPIX], fp8, tag="h1q", name="h1q")
            scr8 = cpool.tile([128, 256], fp8, tag="scr8", name="scr8")
            scro = cpool.tile([128, 128], f32, tag="scro", name="scro")
            out_sb = opool.tile([OUT, NPIX], f32, tag="osb")

            # packed views
            b0_v = w0l_t[:, 0:8].bitcast(f32)          # [128, 2]
            w1_v = wb_t[:, 0:512]
            w2_v = wb_t[:, 512:1024]
            w3_v = wb_t[:, 1024:1030]
            b1_v = wt_t[:, 0:2]
            b2_v = wt_t[:, 2:4]
            b3_v = wt_t[:][0:OUT, 4:5]                 # [3, 1]

            xs3 = xs.rearrange("p (s n) -> p s n", s=3)
            xst3 = xs_t[:].rearrange("p (s n) -> p s n", s=3)

            def slab_rows(ap3, r0, r1):
                return ap3[:, :, r0 * SLABW:r1 * SLABW]

            # ---- input DMAs: deadline order, two HWDGE queues ----
            nc.sync.dma_start(xst3[:, 0:2, 0:6 * SLABW],
                              xs3[:, 0:2, 0:6 * SLABW])          # hi/lo r0-6
            nc.sync.dma_start(w0h_t[:, 0:3200], w0h[:, 0:3200])  # m0
            nc.sync.dma_start(w0l_t[:, 0:8 + WLBLK],
                              w0l[:, 0:8 + WLBLK])               # b0 + m0
            nc.sync.dma_start(xst3[:, 2:3, 0:6 * SLABW],
                              xs3[:, 2:3, 0:6 * SLABW])          # coords r0-6
            nc.sync.dma_start(w0h_t[:, 3200:6400], w0h[:, 3200:6400])  # m1
            nc.sync.dma_start(w0l_t[:, 8 + WLBLK:], w0l[:, 8 + WLBLK:])
            nc.sync.dma_start(slab_rows(xst3, 6, 14), slab_rows(xs3, 6, 14))
            nc.sync.dma_start(slab_rows(xst3, 14, 22), slab_rows(xs3, 14, 22))
            nc.sync.dma_start(slab_rows(xst3, 22, 30), slab_rows(xs3, 22, 30))
            nc.sync.dma_start(slab_rows(xst3, 30, 36), slab_rows(xs3, 30, 36))
            nc.sync.dma_start(wq_t[:], wq[:])
            nc.sync.dma_start(wb_t[:], wb[:])
            nc.sync.dma_start(wt_t[:], wt[:])

            # ---- PE warmup on scratch data (p-state ramp during DMA) ----
            nc.vector.memset(scr8[:], 0.0)
            scr3 = scr8[:].rearrange("p (s n) -> p s n", s=2)
            for i in range(NWARM):
                psw = pspool.tile([128, 128], f32, tag="ps", name=f"psw{i}")
                nc.tensor.matmul(psw[:], scr3, scr3, start=True, stop=True,
                                 perf_mode=DR)
                if i == NWARM - 1:
                    nc.scalar.activation(scro[:], psw[:], Sin,
                                         bias=b0_v[:, 0:1], scale=1.0)

            # ---- fused pipeline ----
            bf16 = mybir.dt.bfloat16
            h0 = hpool.tile([128, 2 * NPIX], bf16, tag="h", name="h0")
            h1 = hpool.tile([128, 2 * NPIX], bf16, tag="h", name="h1")
            h2 = hpool.tile([128, 2 * NPIX], bf16, tag="h", name="h2")
            act_scale = OMEGA / (SX * SWT)

            def w0h_blk(m, k):
                off = m * 3200 + k * 128
                a = w0h_t[:, off:off + 128].unsqueeze(1).copy()
                a.ap[1] = [0, 2]   # stride-0: same hi-weights for both halves
                return a

            def emit_l0_unit(m, y):
                ps = pspool.tile([128, 128], f32, tag="ps",
                                 name=f"ps_l0_{m}_{y}")
                for k in range(25):
                    dy, dx = divmod(k, 5)
                    off = (y + dy) * SLABW + dx
                    nc.tensor.matmul(ps[:], w0h_blk(m, k),
                                     xst3[:, 0:2, off:off + 128],
                                     start=(k == 0), stop=False, perf_mode=DR)
                for pi, (dx, q) in enumerate(WPAIRS):
                    blk = 8 + WLBLK * m + pi * 256
                    lhs = w0l_t[:, blk:blk + 256].rearrange(
                        "p (t c) -> p t c", t=2)
                    off = (y + 2 * q) * SLABW + dx
                    rhs = xst3[:, 0:1, off:off + 128].copy()
                    rhs.ap[1] = [SLABW, 2]       # taps (2q,dx), (2q+1,dx)
                    nc.tensor.matmul(ps[:], lhs, rhs, start=False,
                                     stop=False, perf_mode=DR)
                # mixed pair last: (w_lo of tap12) x window + wcp x coords
                # plane -- tap12's window offset equals the coords window's
                # in-plane offset, so the two-dim stride is exactly 2*SLAB
                blk = 8 + WLBLK * m + 7 * 256
                lhs = w0l_t[:, blk:blk + 256].rearrange(
                    "p (t c) -> p t c", t=2)
                off = (y + 2) * SLABW + 2
                rhs = xst3[:, 0:1, off:off + 128].copy()
                rhs.ap[1] = [2 * SLAB, 2]
                nc.tensor.matmul(ps[:], lhs, rhs, start=False, stop=True,
                                 perf_mode=DR)
                nc.scalar.activation(
                    h0[:, m * NPIX + y * 128:m * NPIX + (y + 1) * 128],
                    ps[:], Sin, bias=b0_v[:, m:m + 1], scale=act_scale)

            def emit_dense(lname, hin, hout, wl_v, bl_v, px0, npx):
                pss = [pspool.tile([128, npx], f32, tag="ps",
                                   name=f"ps_{lname}_{m}_{px0}")
                       for m in range(2)]
                for k in range(2):      # k-major: fresh k=1 read comes last
                    for m in range(2):
                        nc.tensor.matmul(
                            pss[m][:],
                            wl_v[:, (k * 2 + m) * 128:(k * 2 + m + 1) * 128],
                            hin[:, k * NPIX + px0:k * NPIX + px0 + npx],
                            start=(k == 0), stop=(k == 1))
                for m in range(2):
                    nc.scalar.activation(
                        hout[:, m * NPIX + px0:m * NPIX + px0 + npx],
                        pss[m][:], Sin, bias=bl_v[:, m:m + 1], scale=OMEGA)

            def emit_split(hq_t, h_t, k, px0, npx):
                # hq layout: hi at k*4096+px, lo at 8192+k*4096+px
                hi = hq_t[:, k * NPIX + px0:k * NPIX + px0 + npx]
                lo = hq_t[:, 2 * NPIX + k * NPIX + px0:
                          2 * NPIX + k * NPIX + px0 + npx]
                hsl = h_t[:, k * NPIX + px0:k * NPIX + px0 + npx]
                nc.vector.tensor_copy(hi, hsl)
                nc.vector.tensor_sub(lo, hsl, hi)

            def emit_dense_f8(lname, loff, hq_t, hout, bl_v, t):
                # one [128,512] psum bank per m; two 256-px DR chains each
                pss = [pspool.tile([128, TP], f32, tag="ps",
                                   name=f"ps_{lname}f8_{m}_{t}")
                       for m in range(2)]

                def hi_rhs(k, off):
                    a = hq_t[:, k * NPIX + off:k * NPIX + off + 256]
                    a = a.unsqueeze(1).copy()
                    a.ap[1] = [2 * NPIX, 2]        # (hi_k, lo_k)
                    return a

                def wc_rhs(off):
                    a = hq_t[:, off:off + 256].unsqueeze(1).copy()
                    a.ap[1] = [NPIX, 2]            # (hi_k0, hi_k1)
                    return a

                def lhs_main(k, m):
                    a = wq_t[:, loff + (k * 2 + m) * 128:
                             loff + (k * 2 + m) * 128 + 128]
                    a = a.unsqueeze(1).copy()
                    a.ap[1] = [0, 2]
                    return a

                for sub in (0, 256):
                    off = t * TP + sub
                    for m in range(2):
                        o = pss[m][:, sub:sub + 256]
                        nc.tensor.matmul(o, lhs_main(0, m), hi_rhs(0, off),
                                         start=(sub == 0), stop=False,
                                         perf_mode=DR, skip_group_check=True)
                        nc.tensor.matmul(o, lhs_main(1, m), hi_rhs(1, off),
                                         start=False, stop=False,
                                         perf_mode=DR, skip_group_check=True)
                        lw = wq_t[:, loff + 512 + m * 256:
                                  loff + 512 + (m + 1) * 256].rearrange(
                            "p (t c) -> p t c", t=2)
                        nc.tensor.matmul(o, lw, wc_rhs(off), start=False,
                                         stop=(sub == 256), perf_mode=DR,
                                         skip_group_check=True)
                for m in range(2):
                    nc.scalar.activation(
                        hout[:, m * NPIX + t * TP:m * NPIX + (t + 1) * TP],
                        pss[m][:], Sin, bias=bl_v[:, m:m + 1],
                        scale=OMEGA / SW12)

            def emit_head(px0, npx):
                ps = pspool.tile([OUT, npx], f32, tag="ps",
                                 name=f"ps_hd_{px0}")
                for k in range(2):
                    nc.tensor.matmul(
                        ps[:], w3_v[:, k * OUT:(k + 1) * OUT],
                        h2[:, k * NPIX + px0:k * NPIX + px0 + npx],
                        start=(k == 0), stop=(k == 1))
                nc.vector.tensor_scalar_add(
                    out_sb[:, px0:px0 + npx], ps[:], b3_v)

            def out_dma(px0, px1, last=False):
                nc.sync.dma_start(out[:, px0:px1], out_sb[:, px0:px1])

            def l1(px0, npx):
                emit_dense("l1", h0, h1, w1_v, b1_v, px0, npx)

            def l2(px0, npx):
                emit_dense("l2", h1, h2, w2_v, b2_v, px0, npx)

            for i in range(ROWS + 2):
                if i < ROWS:
                    emit_l0_unit(0, i)
                    if i % 2 == 1 and i < 28:
                        emit_split(h0q, h0, 0, (i - 1) * 128, 256)
                if i >= 2:
                    z = i - 2
                    emit_l0_unit(1, z)
                    if z % 2 == 1 and z < 28:
                        emit_split(h0q, h0, 1, (z - 1) * 128, 256)
                    if z % 4 == 3 and z < 28:
                        t = z // 4              # 0..6
                        if t >= 1:
                            emit_dense_f8("l2", 1024, h1q, h2, b2_v, t - 1)
                        if t >= 2:
                            emit_head((t - 2) * TP, TP)
                            if t in (3, 5, 7):
                                out_dma((t - 3) * TP, (t - 1) * TP)
                        emit_dense_f8("l1", 0, h0q, h1, b1_v, t)
                        for m in range(2):
                            emit_split(h1q, h1, m, t * TP, TP)
                    elif z == 28:
                        emit_dense_f8("l2", 1024, h1q, h2, b2_v, 6)
                    elif z == 29:
                        emit_head(5 * TP, TP)
                        out_dma(4 * TP, 6 * TP)
                        l1(3584, 256)           # tile 7 first half
                    elif z == 30:
                        l2(3584, 256)
                        l1(3840, 128)
                    elif z == 31:
                        emit_head(6 * TP, TP)
                        l2(3840, 128)
                        l1(3968, 128)
                        emit_head(3584, 256)
                        emit_head(3840, 128)
                        out_dma(6 * TP, 3968)
                        l2(3968, 128)
                        emit_head(3968, 128)
                        out_dma(3968, NPIX, last=True)

    nc.finalize()
    _BUILT[key] = nc
    return nc


def _to_f32r(a):
    """Round fp32 to the fp32r format the PE expects (low 12 mantissa bits 0)."""
    b = np.ascontiguousarray(a, np.float32).view(np.uint32).astype(np.uint64)
    r = ((b + 0x800) & 0xFFFFF000).astype(np.uint32)
    return r.view(np.float32).reshape(np.asarray(a).shape)


def _e4(a):
    return np.ascontiguousarray(a, np.float32).astype(E4)


def _prep_core_inputs(c, xi, gx, gy):
    b = c // 4
    y0 = (c % 4) * ROWS
    slab = np.zeros((128, SLABR, SLABW), np.float32)
    ylo, yhi = y0 - 2, y0 + ROWS + 2
    slo, shi = max(ylo, 0), min(yhi, H)
    slab[:, slo - ylo: shi - ylo, 2:2 + W] = xi[b, :, slo:shi, :]
    slab *= SX
    xh = _e4(slab)
    xl = _e4(slab - xh.astype(np.float32))

    csl = np.zeros((128, SLABR, SLABW), np.float32)
    csl[0, 2:35, 2:130] = SX * gx[None, :]
    # gy per slab row r (used at window row y'+2 -> image row y0+y'):
    for r in range(2, 35):
        csl[1, r, 2:130] = SX * gy[min(max(y0 + r - 2, 0), H - 1)]

    return {
        "xs": np.concatenate(
            [xh.reshape(128, SLAB), xl.reshape(128, SLAB),
             _e4(csl.reshape(128, SLAB))], axis=1),
    }


def kernel(**inputs):
    from concourse.bass_utils import run_bass_kernel_spmd

    xi = np.asarray(inputs["xi"], np.float32)
    W0 = np.asarray(inputs["W0"], np.float32)
    b0 = np.asarray(inputs["b0"], np.float32)
    W1 = np.asarray(inputs["W1"], np.float32)
    b1 = np.asarray(inputs["b1"], np.float32)
    W2 = np.asarray(inputs["W2"], np.float32)
    b2 = np.asarray(inputs["b2"], np.float32)
    W3 = np.asarray(inputs["W3"], np.float32)
    b3 = np.asarray(inputs["b3"], np.float32)

    # ---- weight prep (replicated) ----
    # patch rows of W0 are (c, dy, dx)-ordered; ktile k=(dy*5+dx) gathers
    # rows c*25+k. Scale 2^12, split hi/lo in e4m3.
    Wp = (SWT * W0[:FC * P * P]).reshape(128, 25, HID)   # [c, k, out]
    wh_f = _e4(Wp).astype(np.float32)
    wl_f = Wp - wh_f
    # coords weight pad: [m][2 halves][128]; half0 rows 0,1 = SWT*Wc
    Wc = SWT * W0[FC * P * P:]                            # [2, 256]
    wcp = np.zeros((128, 2, 2, 128), np.float32)
    for m in range(2):
        wcp[0:2, m, 0, :] = Wc[:, m * 128:(m + 1) * 128]
    # w0h: [m=0 taps][m=1 taps]
    w0h_pk = np.empty((128, 6400), np.float32)
    for m in range(2):
        for k in range(25):
            off = m * 3200 + k * 128
            w0h_pk[:, off:off + 128] = wh_f[:, k, m * 128:(m + 1) * 128]
    # w0l: [b0(8 bytes)][m=0 pair blocks][m=1 pair blocks]
    b0_h = np.ascontiguousarray((OMEGA * b0).reshape(2, 128).T,
                                np.float32)               # [128, 2]
    w0l_pk = np.zeros((128, 8 + 2 * WLBLK), E4)
    w0l_pk[:, 0:8] = b0_h.view(np.uint8).reshape(128, 8).view(E4)
    for m in range(2):
        for pi, (dx, q) in enumerate(WPAIRS):
            for j in range(2):
                k = (2 * q + j) * 5 + dx
                off = 8 + WLBLK * m + pi * 256 + j * 128
                w0l_pk[:, off:off + 128] = _e4(
                    wl_f[:, k, m * 128:(m + 1) * 128])
        off = 8 + WLBLK * m + 7 * 256
        w0l_pk[:, off:off + 128] = _e4(wl_f[:, 12, m * 128:(m + 1) * 128])
        w0l_pk[:, off + 128:off + 256] = _e4(wcp[:, m, 0, :])

    # wb: [w1|w2|w3] bf16; wt: [b1|b2|b3] f32
    wb_pk = np.zeros((128, WBC), ml_dtypes.bfloat16)
    wb_pk[:, 0:512] = W1.reshape(2, 128, 2, 128).transpose(
        1, 0, 2, 3).reshape(128, 512).astype(ml_dtypes.bfloat16)
    wb_pk[:, 512:1024] = W2.reshape(2, 128, 2, 128).transpose(
        1, 0, 2, 3).reshape(128, 512).astype(ml_dtypes.bfloat16)
    wb_pk[:, 1024:1030] = W3.reshape(2, 128, OUT).transpose(
        1, 0, 2).reshape(128, 2 * OUT).astype(ml_dtypes.bfloat16)
    wt_pk = np.zeros((128, WTAIL), np.float32)
    wt_pk[:, 0:2] = np.ascontiguousarray((OMEGA * b1).reshape(2, 128).T)
    wt_pk[:, 2:4] = np.ascontiguousarray((OMEGA * b2).reshape(2, 128).T)
    wt_pk[0:OUT, 4] = b3

    ys = np.linspace(-1.0, 1.0, H, dtype=np.float32)
    xcs = np.linspace(-1.0, 1.0, W, dtype=np.float32)

    # wq: per layer [hi blocks (k,m)][lo pairs (m)] in e4m3, scale 2^6
    wq_pk = np.zeros((128, 2048), E4)
    for li, Wl in ((0, W1), (1, W2)):
        whf = _e4(SW12 * Wl).astype(np.float32)
        wlf = SW12 * Wl - whf
        base = li * 1024
        for k in range(2):
            for m in range(2):
                off = base + (k * 2 + m) * 128
                wq_pk[:, off:off + 128] = _e4(
                    whf[k * 128:(k + 1) * 128, m * 128:(m + 1) * 128])
        for m in range(2):
            for k in range(2):
                off = base + 512 + m * 256 + k * 128
                wq_pk[:, off:off + 128] = _e4(
                    wlf[k * 128:(k + 1) * 128, m * 128:(m + 1) * 128])

    shared = {"w0h": _e4(w0h_pk), "w0l": w0l_pk, "wt": wt_pk,
              "wb": wb_pk, "wq": wq_pk}
    in_maps = []
    for c in range(NCORES):
        m = _prep_core_inputs(c, xi, xcs, ys)
        m.update(shared)
        in_maps.append(m)

    nc = _build()
    res = run_bass_kernel_spmd(nc, in_maps, core_ids=list(range(NCORES)))
    global LAST_RES
    LAST_RES = res

    full = np.empty((B, OUT, H, W), np.float32)
    for c in range(NCORES):
        b = c // 4
        y0 = (c % 4) * ROWS
        full[b, :, y0:y0 + ROWS, :] = res.results[c]["out"].reshape(
            OUT, ROWS, W)
    return full


# revision 33
# speedup vs baseline: 1.0001x; 1.0001x over previous
"""NeRD pixel decoder (SIREN MLP over 5x5 local patches) on 8 trn2 cores.

Sharding: row-shard the pixel dim. Core c handles image b=c//4, rows
y0=(c%4)*32 .. y0+32 (4096 pixels). SIREN weights replicated.

Layer 0 (the 5x5 conv, 84% of FLOPs) runs in fp8-e4m3 DoubleRow matmuls at
0.5 cycles/row: per output row and 128-out-chan block, 25 taps are computed
as DR pairs (x_hi, x_lo) against stride-0-duplicated fp8 weights (x split
into hi + lo e4m3 parts on host, recovering ~11-bit input precision), plus 8
weight-residual correction DR pairs: 7 over vertically adjacent taps (their
windows don't overlap -- overlapping DR rhs windows crash the PE) and one
mixed pair whose halves are (w_lo of tap12) and the coords contribution
(gx/gy baked into a third slab plane at the same in-plane offset as tap12,
so the pair's two-dim stride is exactly 2*SLAB). Layers 1/2 (tiles 0-6)
also run fp8-e4m3 DoubleRow: the DVE splits each bf16 h tile into e4m3
hi/lo planes (cast + subtract, ~0.3us per 256-px slice on an otherwise
idle engine), and each 512-px psum bank takes two 256-px chains of three
DR matmuls (two stride-0 hi/lo k-mains + one w_lo correction pair) --
768 cycles instead of 1024. The head and the drain tile stay in bf16
weights x bf16 activations (1 cycle/row at any moving size, unlike f32r's
4x penalty under 256), which makes the 128-px drain sub-tiles cheap.

Pipeline: ~12 input DMAs (hi/lo/coords slabs ride one tensor in row chunks;
b0 rides w0l via byte-packing; w1/w2/w3 ride one bf16 tensor; b1/b2/b3 one
tiny f32 tensor), all on the SP queue in strict first-use order -- each
dma_start costs ~650ns of sequencer + HWDGE issue and transfers serialize,
so order is everything and fewer is faster. Output DMAs also go on the SP
queue: a dma_start dispatched from the ACT engine blocks later activations
on that sequencer. Dummy fp8 warmup matmuls on a memset scratch cover the
DMA lead-in and the PE p-state ramp (full clock needs ~3us of continuous
execution). m=1 row units lag m=0 by two rows so the m=1 weight DMA can
land later. L1/L2/head tiles are fused into the L0 row stream with a
one-tile stagger, and the final tile drains through 256/128-px sub-tiles
with filler matmuls between pipeline links, so cross-engine dependencies
are old when the PE reaches them and the drain tail is short.

Everything is quantized host-side (e4m3 via ml_dtypes, f32r/bf16 rounding);
the device only multiplies exactly and accumulates in f32 PSUM. Weight
scale 2^12 and x scale 2^2 keep e4m3 operands in normal range; the
activation scale folds 2^-14 back out (sin(OMEGA*(z+b0)) via ACT bias).

Measured on the 8-core axon trn2 setup: TimelineSim 82044 ns (sim matched
HW within 2% on the 147030 ns baseline), rel err 1.83e-2 vs the fp32
reference (gate 2e-2; fully deterministic for the fixed seed-0 inputs).
Dense layers emit k-major so the freshest cross-engine dependency (the
just-written h half) is read by the last matmul of the group, not the
first.
"""

import numpy as np
import ml_dtypes

FC = 128      # feature channels
P = 5         # patch
HID = 256
OUT = 3
OMEGA = 30.0
B, H, W = 2, 128, 128
NCORES = 8
ROWS = H // 4            # 32 image rows per core
NPIX = ROWS * W          # 4096 pixels per core
SLABR = ROWS + 4         # 36 slab rows (2 halo each side)
SLABW = W + 4            # 132 slab cols (2 pad each side)
SLAB = SLABR * SLABW     # 4752
TP = 512                 # pixels per L1/L2/head PSUM tile (= 4 image rows)
NT = NPIX // TP          # 8 tiles per core

E4 = ml_dtypes.float8_e4m3
SX = 4.0                 # x (slab/coords) pre-scale
SWT = 4096.0             # layer-0 weight pre-scale
SW12 = 64.0              # L1/L2 fp8 weight pre-scale
NCORR = 15               # w_lo-corrected taps: 7 vertical pairs + tap12 in the
                         # mixed pair whose second half is the coords plane
NWARM = 72               # warmup DR matmuls during DMA lead-in
WPAIRS = [(dx, q) for q in range(2) for dx in range(5)][:7]  # 14 taps; +tap12 mixed
WLBLK = 8 * 256          # per-m w0l bytes: 8 DR pair blocks
WTAIL = 5                # packed b1|b2|b3 columns (f32)
WBC = 1030               # packed w1|w2|w3 columns (bf16)

_BUILT = {}


def _build(structure="v8"):
    key = structure
    if key in _BUILT:
        return _BUILT[key]

    import concourse.tile as tile
    import concourse.mybir as mybir
    from concourse import bacc

    f32 = mybir.dt.float32
    f32r = mybir.dt.float32r
    fp8 = mybir.dt.float8e4
    Sin = mybir.ActivationFunctionType.Sin
    DR = mybir.MatmulPerfMode.DoubleRow

    nc = bacc.Bacc("TRN2", target_bir_lowering=False, debug=False)

    xs = nc.dram_tensor("xs", [128, 3 * SLAB], fp8, kind="ExternalInput").ap()
    w0h = nc.dram_tensor("w0h", [128, 6400], fp8,
                         kind="ExternalInput").ap()
    w0l = nc.dram_tensor("w0l", [128, 8 + 2 * WLBLK], fp8,
                         kind="ExternalInput").ap()
    wt = nc.dram_tensor("wt", [128, WTAIL], f32, kind="ExternalInput").ap()
    wb = nc.dram_tensor("wb", [128, WBC], mybir.dt.bfloat16,
                        kind="ExternalInput").ap()
    wq = nc.dram_tensor("wq", [128, 2048], fp8, kind="ExternalInput").ap()
    out = nc.dram_tensor("out", [OUT, NPIX], f32, kind="ExternalOutput").ap()

    with tile.TileContext(nc) as tc:
        with (
            tc.tile_pool(name="const", bufs=1) as cpool,
            tc.tile_pool(name="h", bufs=3) as hpool,
            tc.tile_pool(name="osb", bufs=1) as opool,
            tc.tile_pool(name="ps", bufs=8, space="PSUM") as pspool,
        ):
            # ---- SBUF tiles ----
            xs_t = cpool.tile([128, 3 * SLAB], fp8, tag="xs", name="xs_t")
            w0h_t = cpool.tile([128, 6400], fp8, tag="w0h", name="w0h_t")
            w0l_t = cpool.tile([128, 8 + 2 * WLBLK], fp8, tag="w0l",
                               name="w0l_t")
            wt_t = cpool.tile([128, WTAIL], f32, tag="wt", name="wt_t")
            wb_t = cpool.tile([128, WBC], mybir.dt.bfloat16, tag="wb",
                              name="wb_t")
            wq_t = cpool.tile([128, 2048], fp8, tag="wq", name="wq_t")
            h0q = cpool.tile([128, 4 * NPIX], fp8, tag="h0q", name="h0q")
            h1q = cpool.tile([128, 4 * NPIX], fp8, tag="h1q", name="h1q")
            scr8 = cpool.tile([128, 256], fp8, tag="scr8", name="scr8")
            scro = cpool.tile([128, 128], f32, tag="scro", name="scro")
            out_sb = opool.tile([OUT, NPIX], f32, tag="osb")

            # packed views
            b0_v = w0l_t[:, 0:8].bitcast(f32)          # [128, 2]
            w1_v = wb_t[:, 0:512]
            w2_v = wb_t[:, 512:1024]
            w3_v = wb_t[:, 1024:1030]
            b1_v = wt_t[:, 0:2]
            b2_v = wt_t[:, 2:4]
            b3_v = wt_t[:][0:OUT, 4:5]                 # [3, 1]

            xs3 = xs.rearrange("p (s n) -> p s n", s=3)
            xst3 = xs_t[:].rearrange("p (s n) -> p s n", s=3)

            def slab_rows(ap3, r0, r1):
                return ap3[:, :, r0 * SLABW:r1 * SLABW]

            # ---- input DMAs: deadline order, two HWDGE queues ----
            nc.sync.dma_start(xst3[:, 0:2, 0:6 * SLABW],
                              xs3[:, 0:2, 0:6 * SLABW])          # hi/lo r0-6
            nc.sync.dma_start(w0h_t[:, 0:3200], w0h[:, 0:3200])  # m0
            nc.sync.dma_start(w0l_t[:, 0:8 + WLBLK],
                              w0l[:, 0:8 + WLBLK])               # b0 + m0
            nc.sync.dma_start(xst3[:, 2:3, 0:6 * SLABW],
                              xs3[:, 2:3, 0:6 * SLABW])          # coords r0-6
            nc.sync.dma_start(w0h_t[:, 3200:6400], w0h[:, 3200:6400])  # m1
            nc.sync.dma_start(w0l_t[:, 8 + WLBLK:], w0l[:, 8 + WLBLK:])
            nc.sync.dma_start(slab_rows(xst3, 6, 14), slab_rows(xs3, 6, 14))
            nc.sync.dma_start(slab_rows(xst3, 14, 22), slab_rows(xs3, 14, 22))
            nc.sync.dma_start(slab_rows(xst3, 22, 30), slab_rows(xs3, 22, 30))
            nc.sync.dma_start(slab_rows(xst3, 30, 36), slab_rows(xs3, 30, 36))
            nc.sync.dma_start(wq_t[:], wq[:])
            nc.sync.dma_start(wb_t[:], wb[:])
            nc.sync.dma_start(wt_t[:], wt[:])

            # ---- PE warmup on scratch data (p-state ramp during DMA) ----
            nc.vector.memset(scr8[:], 0.0)
            scr3 = scr8[:].rearrange("p (s n) -> p s n", s=2)
            for i in range(NWARM):
                psw = pspool.tile([128, 128], f32, tag="ps", name=f"psw{i}")
                nc.tensor.matmul(psw[:], scr3, scr3, start=True, stop=True,
                                 perf_mode=DR)
                if i == NWARM - 1:
                    nc.scalar.activation(scro[:], psw[:], Sin,
                                         bias=b0_v[:, 0:1], scale=1.0)

            # ---- fused pipeline ----
            bf16 = mybir.dt.bfloat16
            h0 = hpool.tile([128, 2 * NPIX], bf16, tag="h", name="h0")
            h1 = hpool.tile([128, 2 * NPIX], bf16, tag="h", name="h1")
            h2 = hpool.tile([128, 2 * NPIX], bf16, tag="h", name="h2")
            act_scale = OMEGA / (SX * SWT)

            def w0h_blk(m, k):
                off = m * 3200 + k * 128
                a = w0h_t[:, off:off + 128].unsqueeze(1).copy()
                a.ap[1] = [0, 2]   # stride-0: same hi-weights for both halves
                return a

            def emit_l0_unit(m, y):
                ps = pspool.tile([128, 128], f32, tag="ps",
                                 name=f"ps_l0_{m}_{y}")
                for k in range(25):
                    dy, dx = divmod(k, 5)
                    off = (y + dy) * SLABW + dx
                    nc.tensor.matmul(ps[:], w0h_blk(m, k),
                                     xst3[:, 0:2, off:off + 128],
                                     start=(k == 0), stop=False, perf_mode=DR)
                for pi, (dx, q) in enumerate(WPAIRS):
                    blk = 8 + WLBLK * m + pi * 256
                    lhs = w0l_t[:, blk:blk + 256].rearrange(
                        "p (t c) -> p t c", t=2)
                    off = (y + 2 * q) * SLABW + dx
                    rhs = xst3[:, 0:1, off:off + 128].copy()
                    rhs.ap[1] = [SLABW, 2]       # taps (2q,dx), (2q+1,dx)
                    nc.tensor.matmul(ps[:], lhs, rhs, start=False,
                                     stop=False, perf_mode=DR)
                # mixed pair last: (w_lo of tap12) x window + wcp x coords
                # plane -- tap12's window offset equals the coords window's
                # in-plane offset, so the two-dim stride is exactly 2*SLAB
                blk = 8 + WLBLK * m + 7 * 256
                lhs = w0l_t[:, blk:blk + 256].rearrange(
                    "p (t c) -> p t c", t=2)
                off = (y + 2) * SLABW + 2
                rhs = xst3[:, 0:1, off:off + 128].copy()
                rhs.ap[1] = [2 * SLAB, 2]
                nc.tensor.matmul(ps[:], lhs, rhs, start=False, stop=True,
                                 perf_mode=DR)
                nc.scalar.activation(
                    h0[:, m * NPIX + y * 128:m * NPIX + (y + 1) * 128],
                    ps[:], Sin, bias=b0_v[:, m:m + 1], scale=act_scale)

            def emit_dense(lname, hin, hout, wl_v, bl_v, px0, npx):
                pss = [pspool.tile([128, npx], f32, tag="ps",
                                   name=f"ps_{lname}_{m}_{px0}")
                       for m in range(2)]
                for k in range(2):      # k-major: fresh k=1 read comes last
                    for m in range(2):
                        nc.tensor.matmul(
                            pss[m][:],
                            wl_v[:, (k * 2 + m) * 128:(k * 2 + m + 1) * 128],
                            hin[:, k * NPIX + px0:k * NPIX + px0 + npx],
                            start=(k == 0), stop=(k == 1))
                for m in range(2):
                    nc.scalar.activation(
                        hout[:, m * NPIX + px0:m * NPIX + px0 + npx],
                        pss[m][:], Sin, bias=bl_v[:, m:m + 1], scale=OMEGA)

            def emit_split(hq_t, h_t, k, px0, npx):
                # hq layout: hi at k*4096+px, lo at 8192+k*4096+px
                hi = hq_t[:, k * NPIX + px0:k * NPIX + px0 + npx]
                lo = hq_t[:, 2 * NPIX + k * NPIX + px0:
                          2 * NPIX + k * NPIX + px0 + npx]
                hsl = h_t[:, k * NPIX + px0:k * NPIX + px0 + npx]
                nc.vector.tensor_copy(hi, hsl)
                nc.vector.tensor_sub(lo, hsl, hi)

            def emit_dense_f8(lname, loff, hq_t, hout, bl_v, t):
                # one [128,512] psum bank per m; two 256-px DR chains each
                pss = [pspool.tile([128, TP], f32, tag="ps",
                                   name=f"ps_{lname}f8_{m}_{t}")
                       for m in range(2)]

                def hi_rhs(k, off):
                    a = hq_t[:, k * NPIX + off:k * NPIX + off + 256]
                    a = a.unsqueeze(1).copy()
                    a.ap[1] = [2 * NPIX, 2]        # (hi_k, lo_k)
                    return a

                def wc_rhs(off):
                    a = hq_t[:, off:off + 256].unsqueeze(1).copy()
                    a.ap[1] = [NPIX, 2]            # (hi_k0, hi_k1)
                    return a

                def lhs_main(k, m):
                    a = wq_t[:, loff + (k * 2 + m) * 128:
                             loff + (k * 2 + m) * 128 + 128]
                    a = a.unsqueeze(1).copy()
                    a.ap[1] = [0, 2]
                    return a

                for sub in (0, 256):
                    off = t * TP + sub
                    for m in range(2):
                        o = pss[m][:, sub:sub + 256]
                        nc.tensor.matmul(o, lhs_main(0, m), hi_rhs(0, off),
                                         start=(sub == 0), stop=False,
                                         perf_mode=DR, skip_group_check=True)
                        nc.tensor.matmul(o, lhs_main(1, m), hi_rhs(1, off),
                                         start=False, stop=False,
                                         perf_mode=DR, skip_group_check=True)
                        lw = wq_t[:, loff + 512 + m * 256:
                                  loff + 512 + (m + 1) * 256].rearrange(
                            "p (t c) -> p t c", t=2)
                        nc.tensor.matmul(o, lw, wc_rhs(off), start=False,
                                         stop=(sub == 256), perf_mode=DR,
                                         skip_group_check=True)
                for m in range(2):
                    nc.scalar.activation(
                        hout[:, m * NPIX + t * TP:m * NPIX + (t + 1) * TP],
                        pss[m][:], Sin, bias=bl_v[:, m:m + 1],
                        scale=OMEGA / SW12)

            def emit_head(px0, npx):
                ps = pspool.tile([OUT, npx], f32, tag="ps",
                                 name=f"ps_hd_{px0}")
                for k in range(2):
                    nc.tensor.matmul(
                        ps[:], w3_v[:, k * OUT:(k + 1) * OUT],
                        h2[:, k * NPIX + px0:k * NPIX + px0 + npx],
                        start=(k == 0), stop=(k == 1))
                nc.vector.tensor_scalar_add(
                    out_sb[:, px0:px0 + npx], ps[:], b3_v)

            def out_dma(px0, px1, last=False):
                nc.sync.dma_start(out[:, px0:px1], out_sb[:, px0:px1])

            def l1(px0, npx):
                emit_dense("l1", h0, h1, w1_v, b1_v, px0, npx)

            def l2(px0, npx):
                emit_dense("l2", h1, h2, w2_v, b2_v, px0, npx)

            for i in range(ROWS + 2):
                if i < ROWS:
                    emit_l0_unit(0, i)
                    if i % 2 == 1 and i < 28:
                        emit_split(h0q, h0, 0, (i - 1) * 128, 256)
                if i >= 2:
                    z = i - 2
                    emit_l0_unit(1, z)
                    if z % 2 == 1 and z < 28:
                        emit_split(h0q, h0, 1, (z - 1) * 128, 256)
                    if z % 4 == 3 and z < 28:
                        t = z // 4              # 0..6
                        if t >= 1:
                            emit_dense_f8("l2", 1024, h1q, h2, b2_v, t - 1)
                        if t >= 2:
                            emit_head((t - 2) * TP, TP)
                            if t in (3, 5, 7):
                                out_dma((t - 3) * TP, (t - 1) * TP)
                        emit_dense_f8("l1", 0, h0q, h1, b1_v, t)
                        for m in range(2):
                            emit_split(h1q, h1, m, t * TP, TP)
                    elif z == 29:
                        emit_head(5 * TP, TP)
                        emit_dense_f8("l2", 1024, h1q, h2, b2_v, 6)
                        out_dma(4 * TP, 6 * TP)
                        l1(3584, 256)           # tile 7 first half
                    elif z == 30:
                        l2(3584, 256)
                        l1(3840, 128)
                    elif z == 31:
                        emit_head(6 * TP, TP)
                        l2(3840, 128)
                        l1(3968, 128)
                        emit_head(3584, 256)
                        emit_head(3840, 128)
                        out_dma(6 * TP, 3968)
                        l2(3968, 128)
                        emit_head(3968, 128)
                        out_dma(3968, NPIX, last=True)

    nc.finalize()
    _BUILT[key] = nc
    return nc


def _to_f32r(a):
    """Round fp32 to the fp32r format the PE expects (low 12 mantissa bits 0)."""
    b = np.ascontiguousarray(a, np.float32).view(np.uint32).astype(np.uint64)
    r = ((b + 0x800) & 0xFFFFF000).astype(np.uint32)
    return r.view(np.float32).reshape(np.asarray(a).shape)


def _e4(a):
    return np.ascontiguousarray(a, np.float32).astype(E4)


def _prep_core_inputs(c, xi, gx, gy):
    b = c // 4
    y0 = (c % 4) * ROWS
    slab = np.zeros((128, SLABR, SLABW), np.float32)
    ylo, yhi = y0 - 2, y0 + ROWS + 2
    slo, shi = max(ylo, 0), min(yhi, H)
    slab[:, slo - ylo: shi - ylo, 2:2 + W] = xi[b, :, slo:shi, :]
    slab *= SX
    xh = _e4(slab)
    xl = _e4(slab - xh.astype(np.float32))

    csl = np.zeros((128, SLABR, SLABW), np.float32)
    csl[0, 2:35, 2:130] = SX * gx[None, :]
    # gy per slab row r (used at window row y'+2 -> image row y0+y'):
    for r in range(2, 35):
        csl[1, r, 2:130] = SX * gy[min(max(y0 + r - 2, 0), H - 1)]

    return {
        "xs": np.concatenate(
            [xh.reshape(128, SLAB), xl.reshape(128, SLAB),
             _e4(csl.reshape(128, SLAB))], axis=1),
    }


def kernel(**inputs):
    from concourse.bass_utils import run_bass_kernel_spmd

    xi = np.asarray(inputs["xi"], np.float32)
    W0 = np.asarray(inputs["W0"], np.float32)
    b0 = np.asarray(inputs["b0"], np.float32)
    W1 = np.asarray(inputs["W1"], np.float32)
    b1 = np.asarray(inputs["b1"], np.float32)
    W2 = np.asarray(inputs["W2"], np.float32)
    b2 = np.asarray(inputs["b2"], np.float32)
    W3 = np.asarray(inputs["W3"], np.float32)
    b3 = np.asarray(inputs["b3"], np.float32)

    # ---- weight prep (replicated) ----
    # patch rows of W0 are (c, dy, dx)-ordered; ktile k=(dy*5+dx) gathers
    # rows c*25+k. Scale 2^12, split hi/lo in e4m3.
    Wp = (SWT * W0[:FC * P * P]).reshape(128, 25, HID)   # [c, k, out]
    wh_f = _e4(Wp).astype(np.float32)
    wl_f = Wp - wh_f
    # coords weight pad: [m][2 halves][128]; half0 rows 0,1 = SWT*Wc
    Wc = SWT * W0[FC * P * P:]                            # [2, 256]
    wcp = np.zeros((128, 2, 2, 128), np.float32)
    for m in range(2):
        wcp[0:2, m, 0, :] = Wc[:, m * 128:(m + 1) * 128]
    # w0h: [m=0 taps][m=1 taps]
    w0h_pk = np.empty((128, 6400), np.float32)
    for m in range(2):
        for k in range(25):
            off = m * 3200 + k * 128
            w0h_pk[:, off:off + 128] = wh_f[:, k, m * 128:(m + 1) * 128]
    # w0l: [b0(8 bytes)][m=0 pair blocks][m=1 pair blocks]
    b0_h = np.ascontiguousarray((OMEGA * b0).reshape(2, 128).T,
                                np.float32)               # [128, 2]
    w0l_pk = np.zeros((128, 8 + 2 * WLBLK), E4)
    w0l_pk[:, 0:8] = b0_h.view(np.uint8).reshape(128, 8).view(E4)
    for m in range(2):
        for pi, (dx, q) in enumerate(WPAIRS):
            for j in range(2):
                k = (2 * q + j) * 5 + dx
                off = 8 + WLBLK * m + pi * 256 + j * 128
                w0l_pk[:, off:off + 128] = _e4(
                    wl_f[:, k, m * 128:(m + 1) * 128])
        off = 8 + WLBLK * m + 7 * 256
        w0l_pk[:, off:off + 128] = _e4(wl_f[:, 12, m * 128:(m + 1) * 128])
        w0l_pk[:, off + 128:off + 256] = _e4(wcp[:, m, 0, :])

    # wb: [w1|w2|w3] bf16; wt: [b1|b2|b3] f32
    wb_pk = np.zeros((128, WBC), ml_dtypes.bfloat16)
    wb_pk[:, 0:512] = W1.reshape(2, 128, 2, 128).transpose(
        1, 0, 2, 3).reshape(128, 512).astype(ml_dtypes.bfloat16)
    wb_pk[:, 512:1024] = W2.reshape(2, 128, 2, 128).transpose(
        1, 0, 2, 3).reshape(128, 512).astype(ml_dtypes.bfloat16)
    wb_pk[:, 1024:1030] = W3.reshape(2, 128, OUT).transpose(
        1, 0, 2).reshape(128, 2 * OUT).astype(ml_dtypes.bfloat16)
    wt_pk = np.zeros((128, WTAIL), np.float32)
    wt_pk[:, 0:2] = np.ascontiguousarray((OMEGA * b1).reshape(2, 128).T)
    wt_pk[:, 2:4] = np.ascontiguousarray((OMEGA * b2).reshape(2, 128).T)
    wt_pk[0:OUT, 4] = b3

    ys = np.linspace(-1.0, 1.0, H, dtype=np.float32)
    xcs = np.linspace(-1.0, 1.0, W, dtype=np.float32)

    # wq: per layer [hi blocks (k,m)][lo pairs (m)] in e4m3, scale 2^6
    wq_pk = np.zeros((128, 2048), E4)
    for li, Wl in ((0, W1), (1, W2)):
        whf = _e4(SW12 * Wl).astype(np.float32)
        wlf = SW12 * Wl - whf
        base = li * 1024
        for k in range(2):
            for m in range(2):
                off = base + (k * 2 + m) * 128
                wq_pk[:, off:off + 128] = _e4(
                    whf[k * 128:(k + 1) * 128, m * 128:(m + 1) * 128])
        for m in range(2):
            for k in range(2):
                off = base + 512 + m * 256 + k * 128
                wq_pk[:, off:off + 128] = _e4(
                    wlf[k * 128:(k + 1) * 128, m * 128:(m + 1) * 128])

    shared = {"w0h": _e4(w0h_pk), "w0l": w0l_pk, "wt": wt_pk,
              "wb": wb_pk, "wq": wq_pk}
    in_maps = []
    for c in range(NCORES):
        m = _prep_core_inputs(c, xi, xcs, ys)
        m.update(shared)
        in_maps.append(m)

    nc = _build()
    res = run_bass_kernel_spmd(nc, in_maps, core_ids=list(range(NCORES)))
    global LAST_RES
    LAST_RES = res

    full = np.empty((B, OUT, H, W), np.float32)
    for c in range(NCORES):
        b = c // 4
        y0 = (c % 4) * ROWS
        full[b, :, y0:y0 + ROWS, :] = res.results[c]["out"].reshape(
            OUT, ROWS, W)
    return full


# revision 35
# speedup vs baseline: 1.0034x; 1.0033x over previous
"""NeRD pixel decoder (SIREN MLP over 5x5 local patches) on 8 trn2 cores.

Sharding: row-shard the pixel dim. Core c handles image b=c//4, rows
y0=(c%4)*32 .. y0+32 (4096 pixels). SIREN weights replicated.

Layer 0 (the 5x5 conv, 84% of FLOPs) runs in fp8-e4m3 DoubleRow matmuls at
0.5 cycles/row: per output row and 128-out-chan block, 25 taps are computed
as DR pairs (x_hi, x_lo) against stride-0-duplicated fp8 weights (x split
into hi + lo e4m3 parts on host, recovering ~11-bit input precision), plus 8
weight-residual correction DR pairs: 7 over vertically adjacent taps (their
windows don't overlap -- overlapping DR rhs windows crash the PE) and one
mixed pair whose halves are (w_lo of tap12) and the coords contribution
(gx/gy baked into a third slab plane at the same in-plane offset as tap12,
so the pair's two-dim stride is exactly 2*SLAB). Layers 1/2 (tiles 0-6)
also run fp8-e4m3 DoubleRow: the DVE splits each bf16 h tile into e4m3
hi/lo planes (cast + subtract, ~0.3us per 256-px slice on an otherwise
idle engine), and each 512-px psum bank takes two 256-px chains of three
DR matmuls (two stride-0 hi/lo k-mains + one w_lo correction pair) --
768 cycles instead of 1024. The head and the drain tile stay in bf16
weights x bf16 activations (1 cycle/row at any moving size, unlike f32r's
4x penalty under 256), which makes the 128-px drain sub-tiles cheap.

Pipeline: ~12 input DMAs (hi/lo/coords slabs ride one tensor in row chunks;
b0 rides w0l via byte-packing; w1/w2/w3 ride one bf16 tensor; b1/b2/b3 one
tiny f32 tensor), all on the SP queue in strict first-use order -- each
dma_start costs ~650ns of sequencer + HWDGE issue and transfers serialize,
so order is everything and fewer is faster. Output DMAs also go on the SP
queue: a dma_start dispatched from the ACT engine blocks later activations
on that sequencer. Dummy fp8 warmup matmuls on a memset scratch cover the
DMA lead-in and the PE p-state ramp (full clock needs ~3us of continuous
execution). m=1 row units lag m=0 by two rows so the m=1 weight DMA can
land later. L1/L2/head tiles are fused into the L0 row stream with a
one-tile stagger, and the final tile drains through 256/128-px sub-tiles
with filler matmuls between pipeline links, so cross-engine dependencies
are old when the PE reaches them and the drain tail is short.

Everything is quantized host-side (e4m3 via ml_dtypes, f32r/bf16 rounding);
the device only multiplies exactly and accumulates in f32 PSUM. Weight
scale 2^12 and x scale 2^2 keep e4m3 operands in normal range; the
activation scale folds 2^-14 back out (sin(OMEGA*(z+b0)) via ACT bias).

Measured on the 8-core axon trn2 setup: TimelineSim 82033 ns (sim matched
HW within 2% on the 147030 ns baseline), rel err 1.83e-2 vs the fp32
reference (gate 2e-2; fully deterministic for the fixed seed-0 inputs).
Dense layers emit k-major so the freshest cross-engine dependency (the
just-written h half) is read by the last matmul of the group, not the
first.
"""

import numpy as np
import ml_dtypes

FC = 128      # feature channels
P = 5         # patch
HID = 256
OUT = 3
OMEGA = 30.0
B, H, W = 2, 128, 128
NCORES = 8
ROWS = H // 4            # 32 image rows per core
NPIX = ROWS * W          # 4096 pixels per core
SLABR = ROWS + 4         # 36 slab rows (2 halo each side)
SLABW = W + 4            # 132 slab cols (2 pad each side)
SLAB = SLABR * SLABW     # 4752
TP = 512                 # pixels per L1/L2/head PSUM tile (= 4 image rows)
NT = NPIX // TP          # 8 tiles per core

E4 = ml_dtypes.float8_e4m3
SX = 4.0                 # x (slab/coords) pre-scale
SWT = 4096.0             # layer-0 weight pre-scale
SW12 = 64.0              # L1/L2 fp8 weight pre-scale
NCORR = 15               # w_lo-corrected taps: 7 vertical pairs + tap12 in the
                         # mixed pair whose second half is the coords plane
NWARM = 72               # warmup DR matmuls during DMA lead-in
WPAIRS = [(dx, q) for q in range(2) for dx in range(5)][:7]  # 14 taps; +tap12 mixed
WLBLK = 8 * 256          # per-m w0l bytes: 8 DR pair blocks
WTAIL = 5                # packed b1|b2|b3 columns (f32)
WBC = 1030               # packed w1|w2|w3 columns (bf16)

_BUILT = {}


def _build(structure="v8"):
    key = structure
    if key in _BUILT:
        return _BUILT[key]

    import concourse.tile as tile
    import concourse.mybir as mybir
    from concourse import bacc

    f32 = mybir.dt.float32
    f32r = mybir.dt.float32r
    fp8 = mybir.dt.float8e4
    Sin = mybir.ActivationFunctionType.Sin
    DR = mybir.MatmulPerfMode.DoubleRow

    nc = bacc.Bacc("TRN2", target_bir_lowering=False, debug=False)

    xs = nc.dram_tensor("xs", [128, 3 * SLAB], fp8, kind="ExternalInput").ap()
    w0h = nc.dram_tensor("w0h", [128, 6400], fp8,
                         kind="ExternalInput").ap()
    w0l = nc.dram_tensor("w0l", [128, 8 + 2 * WLBLK], fp8,
                         kind="ExternalInput").ap()
    wt = nc.dram_tensor("wt", [128, WTAIL], f32, kind="ExternalInput").ap()
    wb = nc.dram_tensor("wb", [128, WBC], mybir.dt.bfloat16,
                        kind="ExternalInput").ap()
    wq = nc.dram_tensor("wq", [128, 2048], fp8, kind="ExternalInput").ap()
    out = nc.dram_tensor("out", [OUT, NPIX], f32, kind="ExternalOutput").ap()

    with tile.TileContext(nc) as tc:
        with (
            tc.tile_pool(name="const", bufs=1) as cpool,
            tc.tile_pool(name="h", bufs=3) as hpool,
            tc.tile_pool(name="osb", bufs=1) as opool,
            tc.tile_pool(name="ps", bufs=8, space="PSUM") as pspool,
        ):
            # ---- SBUF tiles ----
            xs_t = cpool.tile([128, 3 * SLAB], fp8, tag="xs", name="xs_t")
            w0h_t = cpool.tile([128, 6400], fp8, tag="w0h", name="w0h_t")
            w0l_t = cpool.tile([128, 8 + 2 * WLBLK], fp8, tag="w0l",
                               name="w0l_t")
            wt_t = cpool.tile([128, WTAIL], f32, tag="wt", name="wt_t")
            wb_t = cpool.tile([128, WBC], mybir.dt.bfloat16, tag="wb",
                              name="wb_t")
            wq_t = cpool.tile([128, 2048], fp8, tag="wq", name="wq_t")
            h0q = cpool.tile([128, 4 * NPIX], fp8, tag="h0q", name="h0q")
            h1q = cpool.tile([128, 4 * NPIX], fp8, tag="h1q", name="h1q")
            scr8 = cpool.tile([128, 256], fp8, tag="scr8", name="scr8")
            scro = cpool.tile([128, 128], f32, tag="scro", name="scro")
            out_sb = opool.tile([OUT, NPIX], f32, tag="osb")

            # packed views
            b0_v = w0l_t[:, 0:8].bitcast(f32)          # [128, 2]
            w1_v = wb_t[:, 0:512]
            w2_v = wb_t[:, 512:1024]
            w3_v = wb_t[:, 1024:1030]
            b1_v = wt_t[:, 0:2]
            b2_v = wt_t[:, 2:4]
            b3_v = wt_t[:][0:OUT, 4:5]                 # [3, 1]

            xs3 = xs.rearrange("p (s n) -> p s n", s=3)
            xst3 = xs_t[:].rearrange("p (s n) -> p s n", s=3)

            def slab_rows(ap3, r0, r1):
                return ap3[:, :, r0 * SLABW:r1 * SLABW]

            # ---- input DMAs: deadline order, two HWDGE queues ----
            nc.sync.dma_start(xst3[:, 0:2, 0:6 * SLABW],
                              xs3[:, 0:2, 0:6 * SLABW])          # hi/lo r0-6
            nc.sync.dma_start(w0h_t[:, 0:3200], w0h[:, 0:3200])  # m0
            nc.sync.dma_start(w0l_t[:, 0:8 + WLBLK],
                              w0l[:, 0:8 + WLBLK])               # b0 + m0
            nc.sync.dma_start(xst3[:, 2:3, 0:6 * SLABW],
                              xs3[:, 2:3, 0:6 * SLABW])          # coords r0-6
            nc.sync.dma_start(w0h_t[:, 3200:6400], w0h[:, 3200:6400])  # m1
            nc.sync.dma_start(w0l_t[:, 8 + WLBLK:], w0l[:, 8 + WLBLK:])
            nc.sync.dma_start(slab_rows(xst3, 6, 14), slab_rows(xs3, 6, 14))
            nc.sync.dma_start(slab_rows(xst3, 14, 22), slab_rows(xs3, 14, 22))
            nc.sync.dma_start(slab_rows(xst3, 22, 30), slab_rows(xs3, 22, 30))
            nc.sync.dma_start(slab_rows(xst3, 30, 36), slab_rows(xs3, 30, 36))
            nc.sync.dma_start(wq_t[:], wq[:])
            nc.sync.dma_start(wb_t[:], wb[:])
            nc.sync.dma_start(wt_t[:], wt[:])

            # ---- PE warmup on scratch data (p-state ramp during DMA) ----
            nc.vector.memset(scr8[:], 0.0)
            scr3 = scr8[:].rearrange("p (s n) -> p s n", s=2)
            for i in range(NWARM):
                psw = pspool.tile([128, 128], f32, tag="ps", name=f"psw{i}")
                nc.tensor.matmul(psw[:], scr3, scr3, start=True, stop=True,
                                 perf_mode=DR)
                if i == NWARM - 1:
                    nc.scalar.activation(scro[:], psw[:], Sin,
                                         bias=b0_v[:, 0:1], scale=1.0)

            # ---- fused pipeline ----
            bf16 = mybir.dt.bfloat16
            h0 = hpool.tile([128, 2 * NPIX], bf16, tag="h", name="h0")
            h1 = hpool.tile([128, 2 * NPIX], bf16, tag="h", name="h1")
            h2 = hpool.tile([128, 2 * NPIX], bf16, tag="h", name="h2")
            act_scale = OMEGA / (SX * SWT)

            def w0h_blk(m, k):
                off = m * 3200 + k * 128
                a = w0h_t[:, off:off + 128].unsqueeze(1).copy()
                a.ap[1] = [0, 2]   # stride-0: same hi-weights for both halves
                return a

            def emit_l0_unit(m, y):
                ps = pspool.tile([128, 128], f32, tag="ps",
                                 name=f"ps_l0_{m}_{y}")
                for k in range(25):
                    dy, dx = divmod(k, 5)
                    off = (y + dy) * SLABW + dx
                    nc.tensor.matmul(ps[:], w0h_blk(m, k),
                                     xst3[:, 0:2, off:off + 128],
                                     start=(k == 0), stop=False, perf_mode=DR)
                for pi, (dx, q) in enumerate(WPAIRS):
                    blk = 8 + WLBLK * m + pi * 256
                    lhs = w0l_t[:, blk:blk + 256].rearrange(
                        "p (t c) -> p t c", t=2)
                    off = (y + 2 * q) * SLABW + dx
                    rhs = xst3[:, 0:1, off:off + 128].copy()
                    rhs.ap[1] = [SLABW, 2]       # taps (2q,dx), (2q+1,dx)
                    nc.tensor.matmul(ps[:], lhs, rhs, start=False,
                                     stop=False, perf_mode=DR)
                # mixed pair last: (w_lo of tap12) x window + wcp x coords
                # plane -- tap12's window offset equals the coords window's
                # in-plane offset, so the two-dim stride is exactly 2*SLAB
                blk = 8 + WLBLK * m + 7 * 256
                lhs = w0l_t[:, blk:blk + 256].rearrange(
                    "p (t c) -> p t c", t=2)
                off = (y + 2) * SLABW + 2
                rhs = xst3[:, 0:1, off:off + 128].copy()
                rhs.ap[1] = [2 * SLAB, 2]
                nc.tensor.matmul(ps[:], lhs, rhs, start=False, stop=True,
                                 perf_mode=DR)
                nc.scalar.activation(
                    h0[:, m * NPIX + y * 128:m * NPIX + (y + 1) * 128],
                    ps[:], Sin, bias=b0_v[:, m:m + 1], scale=act_scale)

            def emit_dense(lname, hin, hout, wl_v, bl_v, px0, npx):
                pss = [pspool.tile([128, npx], f32, tag="ps",
                                   name=f"ps_{lname}_{m}_{px0}")
                       for m in range(2)]
                for k in range(2):      # k-major: fresh k=1 read comes last
                    for m in range(2):
                        nc.tensor.matmul(
                            pss[m][:],
                            wl_v[:, (k * 2 + m) * 128:(k * 2 + m + 1) * 128],
                            hin[:, k * NPIX + px0:k * NPIX + px0 + npx],
                            start=(k == 0), stop=(k == 1))
                for m in range(2):
                    nc.scalar.activation(
                        hout[:, m * NPIX + px0:m * NPIX + px0 + npx],
                        pss[m][:], Sin, bias=bl_v[:, m:m + 1], scale=OMEGA)

            def emit_split(hq_t, h_t, k, px0, npx):
                # hq layout: hi at k*4096+px, lo at 8192+k*4096+px
                hi = hq_t[:, k * NPIX + px0:k * NPIX + px0 + npx]
                lo = hq_t[:, 2 * NPIX + k * NPIX + px0:
                          2 * NPIX + k * NPIX + px0 + npx]
                hsl = h_t[:, k * NPIX + px0:k * NPIX + px0 + npx]
                nc.vector.tensor_copy(hi, hsl)
                nc.vector.tensor_sub(lo, hsl, hi)

            def emit_dense_f8(lname, loff, hq_t, hout, bl_v, t):
                # one [128,512] psum bank per m; two 256-px DR chains each
                pss = [pspool.tile([128, TP], f32, tag="ps",
                                   name=f"ps_{lname}f8_{m}_{t}")
                       for m in range(2)]

                def hi_rhs(k, off):
                    a = hq_t[:, k * NPIX + off:k * NPIX + off + 256]
                    a = a.unsqueeze(1).copy()
                    a.ap[1] = [2 * NPIX, 2]        # (hi_k, lo_k)
                    return a

                def wc_rhs(off):
                    a = hq_t[:, off:off + 256].unsqueeze(1).copy()
                    a.ap[1] = [NPIX, 2]            # (hi_k0, hi_k1)
                    return a

                def lhs_main(k, m):
                    a = wq_t[:, loff + (k * 2 + m) * 128:
                             loff + (k * 2 + m) * 128 + 128]
                    a = a.unsqueeze(1).copy()
                    a.ap[1] = [0, 2]
                    return a

                for sub in (0, 256):
                    off = t * TP + sub
                    for m in range(2):
                        o = pss[m][:, sub:sub + 256]
                        nc.tensor.matmul(o, lhs_main(0, m), hi_rhs(0, off),
                                         start=(sub == 0), stop=False,
                                         perf_mode=DR, skip_group_check=True)
                        nc.tensor.matmul(o, lhs_main(1, m), hi_rhs(1, off),
                                         start=False, stop=False,
                                         perf_mode=DR, skip_group_check=True)
                        lw = wq_t[:, loff + 512 + m * 256:
                                  loff + 512 + (m + 1) * 256].rearrange(
                            "p (t c) -> p t c", t=2)
                        nc.tensor.matmul(o, lw, wc_rhs(off), start=False,
                                         stop=(sub == 256), perf_mode=DR,
                                         skip_group_check=True)
                for m in range(2):
                    nc.scalar.activation(
                        hout[:, m * NPIX + t * TP:m * NPIX + (t + 1) * TP],
                        pss[m][:], Sin, bias=bl_v[:, m:m + 1],
                        scale=OMEGA / SW12)

            def emit_head(px0, npx):
                ps = pspool.tile([OUT, npx], f32, tag="ps",
                                 name=f"ps_hd_{px0}")
                for k in range(2):
                    nc.tensor.matmul(
                        ps[:], w3_v[:, k * OUT:(k + 1) * OUT],
                        h2[:, k * NPIX + px0:k * NPIX + px0 + npx],
                        start=(k == 0), stop=(k == 1))
                nc.vector.tensor_scalar_add(
                    out_sb[:, px0:px0 + npx], ps[:], b3_v)

            def out_dma(px0, px1, last=False):
                nc.sync.dma_start(out[:, px0:px1], out_sb[:, px0:px1])

            def l1(px0, npx):
                emit_dense("l1", h0, h1, w1_v, b1_v, px0, npx)

            def l2(px0, npx):
                emit_dense("l2", h1, h2, w2_v, b2_v, px0, npx)

            for i in range(ROWS + 2):
                if i < ROWS:
                    emit_l0_unit(0, i)
                    if i % 2 == 1 and i < 28:
                        emit_split(h0q, h0, 0, (i - 1) * 128, 256)
                if i >= 2:
                    z = i - 2
                    emit_l0_unit(1, z)
                    if z % 2 == 1 and z < 28:
                        emit_split(h0q, h0, 1, (z - 1) * 128, 256)
                    if z % 4 == 3 and z < 28:
                        t = z // 4              # 0..6
                        if t >= 1:
                            emit_dense_f8("l2", 1024, h1q, h2, b2_v, t - 1)
                        if t >= 2:
                            emit_head((t - 2) * TP, TP)
                            if t in (3, 5, 7):
                                out_dma((t - 3) * TP, (t - 1) * TP)
                        emit_dense_f8("l1", 0, h0q, h1, b1_v, t)
                        for m in range(2):
                            emit_split(h1q, h1, m, t * TP, TP)
                    elif z == 29:
                        emit_head(5 * TP, TP)
                        l1(3584, 256)           # tile 7 first half
                        emit_dense_f8("l2", 1024, h1q, h2, b2_v, 6)
                        out_dma(4 * TP, 6 * TP)
                    elif z == 30:
                        l2(3584, 256)
                        l1(3840, 128)
                    elif z == 31:
                        emit_head(6 * TP, TP)
                        l2(3840, 128)
                        l1(3968, 128)
                        emit_head(3584, 256)
                        emit_head(3840, 128)
                        out_dma(6 * TP, 3968)
                        l2(3968, 128)
                        emit_head(3968, 128)
                        out_dma(3968, NPIX, last=True)

    nc.finalize()
    _BUILT[key] = nc
    return nc


def _to_f32r(a):
    """Round fp32 to the fp32r format the PE expects (low 12 mantissa bits 0)."""
    b = np.ascontiguousarray(a, np.float32).view(np.uint32).astype(np.uint64)
    r = ((b + 0x800) & 0xFFFFF000).astype(np.uint32)
    return r.view(np.float32).reshape(np.asarray(a).shape)


def _e4(a):
    return np.ascontiguousarray(a, np.float32).astype(E4)


def _prep_core_inputs(c, xi, gx, gy):
    b = c // 4
    y0 = (c % 4) * ROWS
    slab = np.zeros((128, SLABR, SLABW), np.float32)
    ylo, yhi = y0 - 2, y0 + ROWS + 2
    slo, shi = max(ylo, 0), min(yhi, H)
    slab[:, slo - ylo: shi - ylo, 2:2 + W] = xi[b, :, slo:shi, :]
    slab *= SX
    xh = _e4(slab)
    xl = _e4(slab - xh.astype(np.float32))

    csl = np.zeros((128, SLABR, SLABW), np.float32)
    csl[0, 2:35, 2:130] = SX * gx[None, :]
    # gy per slab row r (used at window row y'+2 -> image row y0+y'):
    for r in range(2, 35):
        csl[1, r, 2:130] = SX * gy[min(max(y0 + r - 2, 0), H - 1)]

    return {
        "xs": np.concatenate(
            [xh.reshape(128, SLAB), xl.reshape(128, SLAB),
             _e4(csl.reshape(128, SLAB))], axis=1),
    }


def kernel(**inputs):
    from concourse.bass_utils import run_bass_kernel_spmd

    xi = np.asarray(inputs["xi"], np.float32)
    W0 = np.asarray(inputs["W0"], np.float32)
    b0 = np.asarray(inputs["b0"], np.float32)
    W1 = np.asarray(inputs["W1"], np.float32)
    b1 = np.asarray(inputs["b1"], np.float32)
    W2 = np.asarray(inputs["W2"], np.float32)
    b2 = np.asarray(inputs["b2"], np.float32)
    W3 = np.asarray(inputs["W3"], np.float32)
    b3 = np.asarray(inputs["b3"], np.float32)

    # ---- weight prep (replicated) ----
    # patch rows of W0 are (c, dy, dx)-ordered; ktile k=(dy*5+dx) gathers
    # rows c*25+k. Scale 2^12, split hi/lo in e4m3.
    Wp = (SWT * W0[:FC * P * P]).reshape(128, 25, HID)   # [c, k, out]
    wh_f = _e4(Wp).astype(np.float32)
    wl_f = Wp - wh_f
    # coords weight pad: [m][2 halves][128]; half0 rows 0,1 = SWT*Wc
    Wc = SWT * W0[FC * P * P:]                            # [2, 256]
    wcp = np.zeros((128, 2, 2, 128), np.float32)
    for m in range(2):
        wcp[0:2, m, 0, :] = Wc[:, m * 128:(m + 1) * 128]
    # w0h: [m=0 taps][m=1 taps]
    w0h_pk = np.empty((128, 6400), np.float32)
    for m in range(2):
        for k in range(25):
            off = m * 3200 + k * 128
            w0h_pk[:, off:off + 128] = wh_f[:, k, m * 128:(m + 1) * 128]
    # w0l: [b0(8 bytes)][m=0 pair blocks][m=1 pair blocks]
    b0_h = np.ascontiguousarray((OMEGA * b0).reshape(2, 128).T,
                                np.float32)               # [128, 2]
    w0l_pk = np.zeros((128, 8 + 2 * WLBLK), E4)
    w0l_pk[:, 0:8] = b0_h.view(np.uint8).reshape(128, 8).view(E4)
    for m in range(2):
        for pi, (dx, q) in enumerate(WPAIRS):
            for j in range(2):
                k = (2 * q + j) * 5 + dx
                off = 8 + WLBLK * m + pi * 256 + j * 128
                w0l_pk[:, off:off + 128] = _e4(
                    wl_f[:, k, m * 128:(m + 1) * 128])
        off = 8 + WLBLK * m + 7 * 256
        w0l_pk[:, off:off + 128] = _e4(wl_f[:, 12, m * 128:(m + 1) * 128])
        w0l_pk[:, off + 128:off + 256] = _e4(wcp[:, m, 0, :])

    # wb: [w1|w2|w3] bf16; wt: [b1|b2|b3] f32
    wb_pk = np.zeros((128, WBC), ml_dtypes.bfloat16)
    wb_pk[:, 0:512] = W1.reshape(2, 128, 2, 128).transpose(
        1, 0, 2, 3).reshape(128, 512).astype(ml_dtypes.bfloat16)
    wb_pk[:, 512:1024] = W2.reshape(2, 128, 2, 128).transpose(
        1, 0, 2, 3).reshape(128, 512).astype(ml_dtypes.bfloat16)
    wb_pk[:, 1024:1030] = W3.reshape(2, 128, OUT).transpose(
        1, 0, 2).reshape(128, 2 * OUT).astype(ml_dtypes.bfloat16)
    wt_pk = np.zeros((128, WTAIL), np.float32)
    wt_pk[:, 0:2] = np.ascontiguousarray((OMEGA * b1).reshape(2, 128).T)
    wt_pk[:, 2:4] = np.ascontiguousarray((OMEGA * b2).reshape(2, 128).T)
    wt_pk[0:OUT, 4] = b3

    ys = np.linspace(-1.0, 1.0, H, dtype=np.float32)
    xcs = np.linspace(-1.0, 1.0, W, dtype=np.float32)

    # wq: per layer [hi blocks (k,m)][lo pairs (m)] in e4m3, scale 2^6
    wq_pk = np.zeros((128, 2048), E4)
    for li, Wl in ((0, W1), (1, W2)):
        whf = _e4(SW12 * Wl).astype(np.float32)
        wlf = SW12 * Wl - whf
        base = li * 1024
        for k in range(2):
            for m in range(2):
                off = base + (k * 2 + m) * 128
                wq_pk[:, off:off + 128] = _e4(
                    whf[k * 128:(k + 1) * 128, m * 128:(m + 1) * 128])
        for m in range(2):
            for k in range(2):
                off = base + 512 + m * 256 + k * 128
                wq_pk[:, off:off + 128] = _e4(
                    wlf[k * 128:(k + 1) * 128, m * 128:(m + 1) * 128])

    shared = {"w0h": _e4(w0h_pk), "w0l": w0l_pk, "wt": wt_pk,
              "wb": wb_pk, "wq": wq_pk}
    in_maps = []
    for c in range(NCORES):
        m = _prep_core_inputs(c, xi, xcs, ys)
        m.update(shared)
        in_maps.append(m)

    nc = _build()
    res = run_bass_kernel_spmd(nc, in_maps, core_ids=list(range(NCORES)))
    global LAST_RES
    LAST_RES = res

    full = np.empty((B, OUT, H, W), np.float32)
    for c in range(NCORES):
        b = c // 4
        y0 = (c % 4) * ROWS
        full[b, :, y0:y0 + ROWS, :] = res.results[c]["out"].reshape(
            OUT, ROWS, W)
    return full
